# revision 1
# baseline (speedup 1.0000x reference)
"""Trainium2 Bass kernel for nn_MfdFC (spherical weighted-Frechet-mean layer).

Math (per row n of N=B*L=1024, all fp32):
  w = col-softmax(w_raw);  X = x[n] (64 points on S^63)
  a(o) <- x0;  3 iterations of:
      D[o,i] = <a_o, x_i>;  f = arccos(D)/sqrt(1-D^2)  (computed via the
      half-angle arctan identity + a custom-DVE quake rsqrt)
      S = w^T * f;  G = S @ X
      coefA[o] = sum_d A*G ; gn2 = sum G^2 - coefA^2   (exact identities)
      a_o <- (cos gn - sinc(gn)*coefA) * a_o + sinc(gn) * G_o
Sharding: data-parallel over rows; core k owns rows [128k, 128(k+1)).

Layout per core: 8 groups x 16 rows. Row r of a group lives at
partition-half r//8 (offset 64*(r//8)) and free-block r%8 (offset 64*(r%8))
of (128 x 512) group tiles. PE does per-row 64^3 matmuls; ACT holds the
single `trig_and_small` table set (Square/Arctan/Sin/Copy); GPSIMD takes
SBUF-only elementwise ops; DVE runs the custom fused ops and reductions.
"""
import math
import numpy as np

f32 = np.float32
FP = None  # mybir.dt.float32, set at import of concourse below

# ---------------------------------------------------------------------------
# constants
C_IN = 64
C_OUT = 64
D_DIM = 64
ROWS_PER_CORE = 128
N_CORES = 8
GROUP_ROWS = 16
N_GROUPS = ROWS_PER_CORE // GROUP_ROWS  # 8
CLIP = float(f32(1.0) - f32(2.0) ** -23)  # 0.99999988
RSQ_C1 = 1.7584694439735017e-30
RSQ_C2 = -2.755803843779718e-20
SHIFT1 = float(np.int32(1).view(f32))  # denormal whose bit pattern is 1
PI = float(f32(math.pi))
HALF_PI = float(f32(math.pi / 2.0))

_COMPILED = {}

# ---------------------------------------------------------------------------
# custom DVE ops

def _register_custom_ops():
    import concourse.dve_ops as dve_ops
    from concourse.dve_ops import DveOp
    from concourse.dve_spec import (
        Spec, Src0, Src1, C0, C1, C2, Zero, One, Bin, AluOp, lower, maxx,
        _has_src1 as has_src1,
    )
    from concourse.dve_uop import DveOpSpec
    from concourse.dve_table_gen import dve_ver_for

    if "ANT_RSQ_F" in dve_ops._SUB_OPCODE_FOR_NAME:
        return {n: op for n, op in ((o.name, o) for o in dve_ops.OPS)
                if n.startswith("ANT_")}

    def _ref_rsq_f(in0, in1, s0, s1, imm2):
        # in0 = u, in1 = float-view of ~(bits(u)>>1) (computed on GPSIMD)
        u = np.asarray(in0, f32)
        nt = np.asarray(in1, f32)
        m1 = (nt * f32(s0)).astype(f32)
        m2 = (m1 * nt).astype(f32)
        m3 = (m2 * f32(s0)).astype(f32)
        t = (m3 * u).astype(f32)
        return ((t + f32(s1)) * nt).astype(f32)

    _m1 = Src1 * C0
    _m3 = (_m1 * Src1) * C0
    RSQ_F = DveOp("ANT_RSQ_F",
                  Spec(body=((_m3 * Src0) + C1) * Src1, reference=_ref_rsq_f),
                  subdim=False, uops_sha={})

    def _ref_rsq_nr(in0, in1, s0, s1, imm2):
        u = np.asarray(in0, f32); y = np.asarray(in1, f32)
        a = (u * y).astype(f32)
        b = (a * y).astype(f32)
        return ((f32(s0) - (b * f32(s1)).astype(f32)) * y).astype(f32)

    RSQ_NR = DveOp("ANT_RSQ_NR",
                   Spec(body=(C0 - ((Src0 * Src1) * Src1) * C1) * Src1,
                        reference=_ref_rsq_nr),
                   subdim=False, uops_sha={})

    def _ref_zs(in0, in1, s0, s1, imm2):
        D = np.asarray(in0, f32); r = np.asarray(in1, f32)
        lt = (D < 0).astype(f32)
        return (((f32(1.0) + f32(s1) * lt).astype(f32) - D) * r).astype(f32)

    ZS_F = DveOp("ANT_ZS_F",
                 Spec(body=((One + (Src0 < Zero) * C1) - Src0) * Src1,
                      reference=_ref_zs),
                 subdim=False, uops_sha={})

    def _ref_ff(in0, in1, s0, s1, imm2):
        th = np.asarray(in0, f32); r = np.asarray(in1, f32)
        lt = (th < 0).astype(f32)
        return (((f32(s0) * lt).astype(f32) + (f32(s1) * th).astype(f32)) * r).astype(f32)

    F_F = DveOp("ANT_F_F",
                Spec(body=(((Src0 < Zero) * C0) + (Src0 * C1)) * Src1,
                     reference=_ref_ff),
                subdim=False, uops_sha={})

    def _ref_gn2(in0, in1, s0, s1, imm2):
        raw = np.asarray(in0, f32); c = np.asarray(in1, f32)
        return np.maximum((raw - (c * c).astype(f32)).astype(f32), f32(s0))

    GN2_F = DveOp("ANT_GN2_F",
                  Spec(body=maxx(Src0 - Src1 * Src1, C0), reference=_ref_gn2),
                  subdim=False, uops_sha={})

    ops = [RSQ_F, RSQ_NR, ZS_F, F_F, GN2_F]
    base = dve_ops._CUSTOM_DVE_ROW_BASE + len(dve_ops.OPS)
    for i, op in enumerate(ops):
        dve_ops._SUB_OPCODE_FOR_NAME[op.name] = base + i
    # pin shas by compiling once
    for trn in ("TRN2",):
        ver = dve_ver_for(trn)
        for op in ops:
            uops = lower(op.spec, ver=ver)
            s = DveOpSpec(name=op.name, opcode=dve_ops.get_dve_sub_opcode(op.name),
                          uops=uops, rd1_en=has_src1(op.spec))
            op.uops_sha[ver] = s.sha(ver)
    dve_ops.OPS.extend(ops)
    dve_ops.CUSTOM_DVE_SPECS.update({op.name: op.spec for op in ops})
    return {op.name: op for op in ops}


# ---------------------------------------------------------------------------
# per-core Bass program

def _row_slices(r):
    p = 64 * (r // 8)
    fb = 64 * (r % 8)
    return p, fb


def build_program(repeat=1, bufs=None, INTERLEAVE=2, rpg=GROUP_ROWS):
    global FP
    B = {"xg": 2, "work": 2, "ab": 3, "red": 2, "ps": 2}
    if bufs:
        B.update(bufs)
    n_groups = ROWS_PER_CORE // rpg
    from contextlib import ExitStack
    import concourse.bass as bass
    import concourse.bacc as bacc
    import concourse.mybir as mybir
    import concourse.tile as tile

    FP = mybir.dt.float32
    AF = mybir.ActivationFunctionType
    ALU = mybir.AluOpType
    AX = mybir.AxisListType

    OPS = _register_custom_ops()
    RSQ_F, RSQ_NR, ZS_F, F_F, GN2_F = (
        OPS["ANT_RSQ_F"], OPS["ANT_RSQ_NR"], OPS["ANT_ZS_F"],
        OPS["ANT_F_F"], OPS["ANT_GN2_F"])


    INT32 = mybir.dt.int32

    def emit_rsqrt(pool, u_t, shape, tag, nr=True):
        """r = rsqrt(u): DVE shift/xor seed + custom quake op (+Newton)."""
        seed = pool.tile(shape, FP, tag=tag + "_sd")
        nc.vector.tensor_scalar(seed[:, :].bitcast(INT32),
                                u_t[:, :].bitcast(INT32), 1, -1,
                                ALU.logical_shift_right, ALU.bitwise_xor)
        y_t = pool.tile(shape, FP, tag=tag + "_y")
        nc.vector._custom_dve(RSQ_F, out=y_t[:, :], in0=u_t[:, :],
                              in1=seed[:, :], s0=RSQ_C1, s1=RSQ_C2)
        if not nr:
            return y_t
        r_t = pool.tile(shape, FP, tag=tag + "_r")
        nc.vector._custom_dve(RSQ_NR, out=r_t[:, :], in0=u_t[:, :],
                              in1=y_t[:, :], s0=1.5, s1=0.5)
        return r_t

    nc = bacc.Bacc()
    x_d = nc.dram_tensor("x_shard", (ROWS_PER_CORE, C_IN, D_DIM), FP,
                         kind="ExternalInput")
    w_d = nc.dram_tensor("w_mat", (C_IN, C_OUT), FP, kind="ExternalInput")
    id_d = nc.dram_tensor("ident", (64, 64), FP, kind="ExternalInput")
    out_d = nc.dram_tensor("out_shard", (ROWS_PER_CORE, C_OUT, D_DIM), FP,
                           kind="ExternalOutput")

    ctx = ExitStack()
    with ctx:
        tc = ctx.enter_context(tile.TileContext(nc))
        const = ctx.enter_context(tc.tile_pool(name="const", bufs=1))
        xg_p = ctx.enter_context(tc.tile_pool(name="xg", bufs=B["xg"]))
        work = ctx.enter_context(tc.tile_pool(name="work", bufs=B["work"]))
        ab_p = ctx.enter_context(tc.tile_pool(name="ab", bufs=B["ab"]))
        red_p = ctx.enter_context(tc.tile_pool(name="red", bufs=B["red"]))
        psum = ctx.enter_context(tc.tile_pool(name="ps", bufs=B["ps"],
                                              space="PSUM"))

        R = rpg         # rows per group, all at partitions 0-63
        W = 64 * R      # free elems per group tile

        # ---- constants
        w_g = const.tile([128, W], FP, tag="wg")
        for r in range(R):
            nc.sync.dma_start(w_g[0:64, 64 * r:64 * r + 64], w_d[:, :])
            nc.sync.dma_start(w_g[64:128, 64 * r:64 * r + 64], w_d[:, :])
        ident = const.tile([128, 64], FP, tag="ident")
        nc.sync.dma_start(ident[0:64, :], id_d[:, :])
        nc.sync.dma_start(ident[64:128, :], id_d[:, :])
        ones = const.tile([64, 64], FP, tag="ones")
        nc.vector.memset(ones[:, :], 1.0)
        halfpi = const.tile([128, 1], FP, tag="halfpi")
        nc.vector.memset(halfpi[:, :], HALF_PI)

        def b3(t):  # (64, W) -> (64, R, 64) view
            return t[:, :].rearrange("p (j d) -> p j d", d=64)

        def emit_load(st):
            """Load a PAIR of row-groups: A -> partitions 0-63, B -> 64-127."""
            n0a, n0b = st["n0a"], st["n0b"]
            X = xg_p.tile([128, W], FP, tag="xg")
            nc.sync.dma_start(
                X[0:64, :].rearrange("p (j d) -> p j d", d=64),
                x_d[n0a:n0a + R].rearrange("j i d -> i j d"))
            nc.sync.dma_start(
                X[64:128, :].rearrange("p (j d) -> p j d", d=64),
                x_d[n0b:n0b + R].rearrange("j i d -> i j d"))
            XT = {}
            for h, base in (("a", 0), ("b", 64)):
                tp = psum.tile([64, W], FP, tag="tp")
                for r in range(R):
                    nc.tensor.transpose(tp[:, 64 * r:64 * r + 64],
                                        X[base:base + 64, 64 * r:64 * r + 64],
                                        ident[base:base + 64, :])
                XT[h] = xg_p.tile([64, W], FP, tag="xt" + h, name="xt" + h)
                nc.scalar.copy(XT[h][:, :], tp[:, :])
            st["X"] = X
            st["XT"] = XT

        def lift(dst_up, src64, tag_unused=None):
            # SBUF->SBUF DMA moving a (64, *) tile into partitions 64-127
            nc.sync.dma_start(dst_up, src64[:, :])

        def fchain(Dc, shape, tagp):
            """factor chain on a stacked tile: returns S-factor f (same shape)."""
            q = work.tile(shape, FP, tag=tagp + "q")
            nc.scalar.activation(q[:, :], Dc[:, :], AF.Square)
            u = work.tile(shape, FP, tag=tagp + "u")
            nc.vector.tensor_scalar(u[:, :], q[:, :], -1.0, 1.0,
                                    ALU.mult, ALU.add)
            rr = emit_rsqrt(work, u, shape, tagp + "r", nr=False)
            zs = work.tile(shape, FP, tag=tagp + "z")
            nc.vector._custom_dve(ZS_F, out=zs[:, :], in0=Dc[:, :],
                                  in1=rr[:, :], s1=-2.0)
            th = work.tile(shape, FP, tag=tagp + "t")
            nc.scalar.activation(th[:, :], zs[:, :], AF.Arctan)
            f = work.tile(shape, FP, tag=tagp + "f")
            nc.vector._custom_dve(F_F, out=f[:, :], in0=th[:, :],
                                  in1=rr[:, :], s0=PI, s1=2.0)
            return f

        def emit_factor(st, it):
            X, XT = st["X"], st["XT"]
            if it == 0:
                Dc0 = red_p.tile([128, R], FP, tag="dc0")
                for h, base in (("a", 0), ("b", 64)):
                    dcol = psum.tile([64, R], FP, tag="mm")
                    for r in range(R):
                        nc.tensor.matmul(dcol[:, r:r + 1],
                                         XT[h][:, 64 * r:64 * r + 64],
                                         XT[h][:, 64 * r:64 * r + 1])
                    if base == 0:
                        nc.vector.tensor_scalar(Dc0[0:64, :], dcol[:, :],
                                                CLIP, -CLIP, ALU.min, ALU.max)
                    else:
                        tmp = red_p.tile([64, R], FP, tag="dc0t")
                        nc.vector.tensor_scalar(tmp[:, :], dcol[:, :],
                                                CLIP, -CLIP, ALU.min, ALU.max)
                        lift(Dc0[64:128, :], tmp)
                f0 = fchain(Dc0, [128, R], "f0")
                S = work.tile([128, W], FP, tag="sg")
                for r in range(R):
                    nc.vector.tensor_scalar(S[:, 64 * r:64 * r + 64],
                                            w_g[:, 64 * r:64 * r + 64],
                                            f0[:, r:r + 1], None, ALU.mult)
                A = ab_p.tile([128, W], FP, tag="ag")
                for (n0, sl) in ((st["n0a"], slice(0, 64)),
                                 (st["n0b"], slice(64, 128))):
                    nc.sync.dma_start(
                        A[sl, :].rearrange("p (j d) -> p j d", d=64),
                        x_d[n0:n0 + R, 0:1, :].rearrange("j o d -> o j d")
                        .broadcast_to([64, R, 64]))
                st["A"] = A
            else:
                AT = st["AT"]
                Dc = work.tile([128, W], FP, tag="dcf")
                for h, base in (("a", 0), ("b", 64)):
                    dt = psum.tile([64, W], FP, tag="mm")
                    for r in range(R):
                        nc.tensor.matmul(dt[:, 64 * r:64 * r + 64],
                                         XT[h][:, 64 * r:64 * r + 64],
                                         AT[h][:, 64 * r:64 * r + 64])
                    if base == 0:
                        nc.vector.tensor_scalar(Dc[0:64, :], dt[:, :],
                                                CLIP, -CLIP, ALU.min, ALU.max)
                    else:
                        tmp = work.tile([64, W], FP, tag="dcft")
                        nc.vector.tensor_scalar(tmp[:, :], dt[:, :],
                                                CLIP, -CLIP, ALU.min, ALU.max)
                        lift(Dc[64:128, :], tmp)
                ff = fchain(Dc, [128, W], "ff")
                S = work.tile([128, W], FP, tag="sg")
                nc.vector.tensor_tensor(S[:, :], w_g[:, :], ff[:, :], ALU.mult)
            st["S"] = S

        def b3s(t):
            return t[:, :].rearrange("p (j d) -> p j d", d=64)

        def emit_update(st, it):
            X, S, A = st["X"], st["S"], st["A"]
            gsb = work.tile([128, W], FP, tag="gsb")
            for h, base in (("a", 0), ("b", 64)):
                gp = psum.tile([64, W], FP, tag="mm")
                for r in range(R):
                    nc.tensor.matmul(gp[:, 64 * r:64 * r + 64],
                                     S[base:base + 64, 64 * r:64 * r + 64],
                                     X[base:base + 64, 64 * r:64 * r + 64])
                if base == 0:
                    nc.scalar.copy(gsb[0:64, :], gp[:, :])
                else:
                    tmp = work.tile([64, W], FP, tag="gsbt")
                    nc.scalar.copy(tmp[:, :], gp[:, :])
                    lift(gsb[64:128, :], tmp)
            prod = work.tile([128, W], FP, tag="scr1")
            nc.vector.tensor_tensor(prod[:, :], A[:, :], gsb[:, :], ALU.mult)
            coefA = red_p.tile([128, R], FP, tag="coef")
            nc.vector.tensor_reduce(coefA[:, :], b3s(prod), AX.X, ALU.add)
            g2 = work.tile([128, W], FP, tag="scr2")
            nc.scalar.activation(g2[:, :], gsb[:, :], AF.Square)
            gn2r = red_p.tile([128, R], FP, tag="gn2r")
            nc.vector.tensor_reduce(gn2r[:, :], b3s(g2), AX.X, ALU.add)
            gn2 = red_p.tile([128, R], FP, tag="gn2")
            nc.vector._custom_dve(GN2_F, out=gn2[:, :], in0=gn2r[:, :],
                                  in1=coefA[:, :], s0=1e-30)
            rg = emit_rsqrt(red_p, gn2, [128, R], "rg")
            gn = red_p.tile([128, R], FP, tag="gn")
            nc.vector.tensor_tensor(gn[:, :], gn2[:, :], rg[:, :], ALU.mult)
            cosg = red_p.tile([128, R], FP, tag="cosg")
            nc.scalar.activation(cosg[:, :], gn[:, :], AF.Sin,
                                 bias=halfpi[:, 0:1])
            s1t = red_p.tile([128, R], FP, tag="s1t")
            nc.scalar.activation(s1t[:, :], gn[:, :], AF.Sin)
            sc = red_p.tile([128, R], FP, tag="sc")
            nc.vector.tensor_tensor(sc[:, :], s1t[:, :], rg[:, :], ALU.mult)
            t9 = red_p.tile([128, R], FP, tag="t9")
            nc.vector.tensor_tensor(t9[:, :], sc[:, :], coefA[:, :], ALU.mult)
            alpha = red_p.tile([128, R], FP, tag="alpha")
            nc.vector.tensor_tensor(alpha[:, :], cosg[:, :], t9[:, :],
                                    ALU.subtract)
            sc_b = sc[:, :].rearrange("p (j o) -> p j o", o=1)\
                .broadcast_to([128, R, 64])
            al_b = alpha[:, :].rearrange("p (j o) -> p j o", o=1)\
                .broadcast_to([128, R, 64])
            t2 = work.tile([128, W], FP, tag="scr1")
            nc.vector.tensor_tensor(b3s(t2), b3s(gsb), sc_b, ALU.mult)
            t1 = work.tile([128, W], FP, tag="scr2")
            nc.vector.tensor_tensor(b3s(t1), b3s(A), al_b, ALU.mult)
            An = ab_p.tile([128, W], FP, tag="ag")
            nc.vector.tensor_tensor(An[:, :], t1[:, :], t2[:, :], ALU.add)
            st["A"] = An
            if it < 2:
                AT = {}
                for h, base in (("a", 0), ("b", 64)):
                    tpa = psum.tile([64, W], FP, tag="tp")
                    for r in range(R):
                        nc.tensor.transpose(
                            tpa[:, 64 * r:64 * r + 64],
                            An[base:base + 64, 64 * r:64 * r + 64],
                            ident[base:base + 64, :])
                    AT[h] = ab_p.tile([64, W], FP, tag="at" + h, name="at" + h)
                    nc.scalar.copy(AT[h][:, :], tpa[:, :])
                st["AT"] = AT
            else:
                nc.sync.dma_start(
                    out_d[st["n0a"]:st["n0a"] + R].rearrange("j o d -> o j d"),
                    An[0:64, :].rearrange("p (j d) -> p j d", d=64))
                nc.sync.dma_start(
                    out_d[st["n0b"]:st["n0b"] + R].rearrange("j o d -> o j d"),
                    An[64:128, :].rearrange("p (j d) -> p j d", d=64))

        n_pairs = n_groups // 2
        for rep in range(repeat):
            for p0 in range(0, n_pairs, INTERLEAVE):
                sts = []
                for p in range(p0, min(p0 + INTERLEAVE, n_pairs)):
                    sts.append({"n0a": rpg * (2 * p), "n0b": rpg * (2 * p + 1)})
                for st in sts:
                    emit_load(st)
                for it in range(3):
                    for st in sts:
                        emit_factor(st, it)
                    for st in sts:
                        emit_update(st, it)
    nc.compile()
    return nc


# ---------------------------------------------------------------------------
# host entry point

def _get_program():
    if "nc" not in _COMPILED:
        _COMPILED["nc"] = build_program()
    return _COMPILED["nc"]


def kernel(x, w_raw, _trace=False):
    from concourse.bass_utils import run_bass_kernel_spmd
    if _trace:
        try:
            import antenv.axon_hooks  # noqa: F401
        except Exception:
            _trace = False

    x = np.ascontiguousarray(np.asarray(x, f32))
    w_raw = np.asarray(w_raw, f32)
    B, L, C_in, d = x.shape
    N = B * L
    w = np.exp((w_raw - f32(np.log(C_in))).astype(f32)).astype(f32)
    w = (w / w.sum(axis=0, keepdims=True)).astype(f32)
    ident = np.eye(64, dtype=f32)

    xr = x.reshape(N, C_in, d)
    nc = _get_program()
    in_maps = []
    for k in range(N_CORES):
        in_maps.append({
            "x_shard": xr[k * ROWS_PER_CORE:(k + 1) * ROWS_PER_CORE],
            "w_mat": w,
            "ident": ident,
        })
    res = run_bass_kernel_spmd(nc, in_maps, core_ids=list(range(N_CORES)),
                               trace=_trace)
    out = np.concatenate([res.results[k]["out_shard"] for k in range(N_CORES)],
                         axis=0)
    if _trace:
        kernel.last_exec_time_ns = res.exec_time_ns
        kernel.last_results = res
    return out.reshape(B, L, C_OUT, d)

